# revision 42
# baseline (speedup 1.0000x reference)
"""EdgeDecoder kernel for 8 Trainium2 NeuronCores.

Math: out[e] = dot(x_src[i0[e]], w_src) + dot(x_dst[i1[e]], w_dst) + bias.
Rewritten as per-node scores s[n] = x_src[n]@w_src + bias, d[n] = x_dst[n]@w_dst,
then out[e] = s[i0[e]] + d[i1[e]].

Device pipeline (launch 1, per core, per side):
  - Host packs each core's ~250k edges into 128*G tiles of F=40 slots; a
    tile holds edges of at most TWO nodes (free pairing, ~98% fill).
  - Host stages x per tile-node as bf16 [h=128, half, g, m=128]: the two
    nodes of tile (p, g) sit in matmul chunk (half=0, g) and (half=1, g),
    column p.  Phase A is pure PE: 2*G chunk-stationary matmuls
    (lhsT = x^T chunk, rhs = w as [128,1]) land both per-tile endpoint
    scores in PSUM [128, 2, G] - already in window order, so there is no
    score table, no DRAM round-trip and no indirect gather at all.
  - One ACT copy (f32->bf16) -> W [128, 2, G]; dif = W1-W0 (DVE).
  - Per-edge value is a lerp  g = W0 + off*(W1-W0)  with host-shipped
    off in {0,1} (bf16 [128, F, G]): two bf16 DVE passes in 2x_1p mode.
Launch 2 adds the two host-realigned halves (device arithmetic only; the
host only permutes/casts between launches).
"""

import numpy as np
import ml_dtypes

BF16 = ml_dtypes.bfloat16

N_NODES = 100000
HIDDEN = 128
N_EDGES = 2000000
N_CORES = 8
NS = N_NODES // N_CORES         # 12500 nodes per core
F = 40                          # edge slots per tile
G = 51                          # tiles per partition per side (data needs 50)
SG = 2 * G                      # both sides' tile columns, s then d
NB = 8                          # PSUM bank rotation for matmul outputs
NCOL = [(G - k + NB - 1) // NB for k in range(NB)]  # cols per bank tile
OFFC = np.concatenate([[0], np.cumsum(NCOL)])[:NB]
# matmul emission index m (bank m%NB) -> device w column within the side
M2C = np.array([OFFC[m % NB] + m // NB for m in range(G)])
SLOT = 128 * F * SG             # g01 slots per core (both sides)
PER = N_EDGES // N_CORES        # 250000 edges per launch-2 core
COLS = (PER + 127) // 128       # 1954
E_OUT = COLS * 128              # 250112 padded launch-2 edges per core

_CACHE = {}


def _mybir():
    import concourse.mybir as mybir
    return mybir


def _build_launch1(reps=1):
    from contextlib import ExitStack
    import concourse.bacc as bacc
    import concourse.tile as tile
    mybir = _mybir()
    f32 = mybir.dt.float32
    bf16 = mybir.dt.bfloat16

    nc = bacc.Bacc("TRN2", debug=False, num_devices=N_CORES)
    xs = nc.dram_tensor("xs", [128, 2, G, 128], bf16, kind="ExternalInput")
    xd = nc.dram_tensor("xd", [128, 2, G, 128], bf16, kind="ExternalInput")
    wv = nc.dram_tensor("wv", [128, 2], bf16, kind="ExternalInput")
    biasr = nc.dram_tensor("biasr", [128, 1], f32, kind="ExternalInput")
    offb = nc.dram_tensor("offb", [128, F, SG], bf16, kind="ExternalInput")
    g01 = nc.dram_tensor("g01", [128, F, SG], bf16, kind="ExternalOutput")

    add = mybir.AluOpType.add
    mult = mybir.AluOpType.mult
    sub = mybir.AluOpType.subtract
    is_ge = mybir.AluOpType.is_ge
    XCH = 17  # g-columns per x-load DMA (3 loads per half)

    with tile.TileContext(nc) as tc:
        with tc.tile_pool(name="const", bufs=1) as cp, \
             tc.tile_pool(name="xload", bufs=4) as xp, \
             tc.tile_pool(name="work", bufs=2) as wp, \
             tc.tile_pool(name="psum", bufs=1, space="PSUM") as pp:

            wv_t = cp.tile([128, 2], bf16)
            nc.sync.dma_start(out=wv_t[:], in_=wv.ap()[:, :])
            bias_t = cp.tile([128, 1], f32, name="bias_t")
            nc.sync.dma_start(out=bias_t[:], in_=biasr.ap()[:, :])

            _loop = ExitStack()
            if reps > 1:
                _loop.enter_context(
                    tc.For_i(0, reps, 1,
                             hint_engines=(mybir.EngineType.PE,)))

            # scores for both sides land in one [128, 2, SG] tile; column
            # sidx*G + g is tile (.,g) of that side, row 'half' its A/B node
            w = wp.tile([128, 2, SG], bf16, name="w_t", tag="w")
            offt = wp.tile([128, F, SG], bf16, name="offt_t", tag="offt")
            nc.scalar.dma_start(out=offt[:], in_=offb.ap()[:, :, :])

            def side(x, wcol, sidx, nm, use_bias):
                # phase A: per-tile endpoint scores, in window order.
                # Matmul m writes PSUM bank m%NB, column m//NB: consecutive
                # matmuls never serialize on same-bank writeback.  Bank k
                # maps to the contiguous w columns [OFFC[k], OFFC[k]+NCOL[k])
                # (host stages everything in that renumbered column order).
                pst = [pp.tile([128, 2, NCOL[k]], f32, name=f"ps_{nm}{k}",
                               tag=f"ps{k}") for k in range(NB)]
                for half in range(2):
                    for c0 in range(0, G, XCH):
                        c1 = min(c0 + XCH, G)
                        xt = xp.tile([128, XCH, 128], bf16,
                                     name=f"xt_{nm}{half}{c0}", tag="xt")
                        nc.sync.dma_start(
                            out=xt[:, :c1 - c0, :],
                            in_=x.ap()[:, half, c0:c1, :])
                        for j in range(c1 - c0):
                            m = c0 + j
                            nc.tensor.matmul(
                                pst[m % NB][:, half, m // NB:m // NB + 1],
                                xt[:, j, :],
                                wv_t[:, wcol:wcol + 1])
                for k in range(NB):
                    o0 = sidx * G + int(OFFC[k])
                    nc.scalar.activation(
                        out=w[:, :, o0:o0 + NCOL[k]],
                        in_=pst[k][:, :, :],
                        func=mybir.ActivationFunctionType.Copy)
                if use_bias:
                    nc.vector.tensor_scalar_add(
                        out=w[:, :, sidx * G:(sidx + 1) * G],
                        in0=w[:, :, sidx * G:(sidx + 1) * G],
                        scalar1=bias_t[:, :])

            side(xs, 0, 0, "s", True)
            side(xd, 1, 1, "d", False)

            # phase B over both sides at once: lerp select with the
            # host-shipped step mask
            dif = wp.tile([128, SG], bf16, name="dif_t", tag="dif")
            nc.vector.tensor_tensor(
                out=dif[:], in0=w[:, 1, :], in1=w[:, 0, :], op=sub)
            prod = wp.tile([128, F, SG], bf16, name="prod_t", tag="prod")
            nc.vector.tensor_tensor(
                out=prod[:],
                in0=offt[:],
                in1=dif[:].rearrange("p g -> p () g").to_broadcast(
                    [128, F, SG]),
                op=mult)
            gt = wp.tile([128, F, SG], bf16, name="gt_t", tag="gt")
            nc.vector.tensor_tensor(
                out=gt[:],
                in0=prod[:],
                in1=w[:, 0, :].rearrange("p g -> p () g").to_broadcast(
                    [128, F, SG]),
                op=add)
            nc.scalar.dma_start(out=g01.ap()[:, :, :], in_=gt[:])
            _loop.close()

    nc.compile()
    return nc


def _build_launch2(reps=1):
    from contextlib import ExitStack
    import concourse.bacc as bacc
    import concourse.tile as tile
    mybir = _mybir()
    bf16 = mybir.dt.bfloat16

    nc = bacc.Bacc("TRN2", debug=False, num_devices=N_CORES)
    a01 = nc.dram_tensor("a01", [128, 2, COLS], bf16, kind="ExternalInput")
    o = nc.dram_tensor("o", [128, COLS], bf16, kind="ExternalOutput")
    with tile.TileContext(nc) as tc:
        with tc.tile_pool(name="io", bufs=3) as io:
            _loop = ExitStack()
            if reps > 1:
                _loop.enter_context(tc.For_i(0, reps, 1))
            t0 = io.tile([128, 2, COLS], bf16, name="t0", tag="t0")
            to = io.tile([128, COLS], bf16, name="to", tag="to")
            nc.sync.dma_start(out=t0[:], in_=a01.ap()[:, :, :])
            nc.vector.tensor_tensor(out=to[:], in0=t0[:, 0, :],
                                    in1=t0[:, 1, :],
                                    op=mybir.AluOpType.add)
            nc.scalar.dma_start(out=o.ap()[:, :], in_=to[:])
            _loop.close()
    nc.compile()
    return nc


def _prep_side(iarr, side):
    """Per-core: pack edges into F-slot tiles of at most 2 nodes each
    (big nodes split into full tiles; leftovers two-pointer paired).

    Returns nodesAB [CORES,2,128,G] i64 (local node per tile half,
    indexed by matmul emission index m), off [CORES,128,F,G] bf16
    (indexed by device column c=M2C[m]; 1.0 on slots holding the B
    node's edges), pos [E] i64 (slot of edge e in its core's combined
    g01 output: p*(F*SG) + f*SG + side*G + c)."""
    E = iarr.shape[0]
    nodesAB = np.zeros((N_CORES, 2, 128, G), np.int64)
    off = np.zeros((N_CORES, 128, F, G), BF16)
    pos = np.empty(E, np.int64)
    one = BF16(1.0)
    for c in range(N_CORES):
        sel = np.nonzero((iarr >= c * NS) & (iarr < (c + 1) * NS))[0]
        li = iarr[sel] - c * NS
        so = np.argsort(li, kind="stable")
        sli = li[so]
        sedge = sel[so]
        counts = np.bincount(sli, minlength=NS)
        starts = np.concatenate([[0], np.cumsum(counts)])
        # tiles: (nodeA, sliceA, nodeB, sliceB)
        tiles = []
        rem = []  # (count, node, start_index)
        for n in range(NS):
            cnt = int(counts[n])
            st = int(starts[n])
            nfull = cnt // F
            for k in range(nfull):
                tiles.append((n, st + k * F, F, n, 0, 0))
            r = cnt % F
            if r:
                rem.append((r, n, st + nfull * F))
        rem.sort()
        i, j = 0, len(rem) - 1
        while i <= j:
            ra, na, sa = rem[j]
            if i < j and ra + rem[i][0] <= F:
                rb, nb, sbst = rem[i]
                tiles.append((na, sa, ra, nb, sbst, rb))
                i += 1
                j -= 1
            else:
                tiles.append((na, sa, ra, na, 0, 0))
                j -= 1
        if len(tiles) > 128 * G:
            raise RuntimeError(
                f"tile capacity exceeded on core {c}: {len(tiles)}")
        for t, (na, sa, cna, nb, sbst, cb) in enumerate(tiles):
            p, m = t % 128, t // 128
            col = int(M2C[m])
            nodesAB[c, 0, p, m] = na
            nodesAB[c, 1, p, m] = nb
            base = p * (F * SG) + side * G + col
            eA = sedge[sa:sa + cna]
            pos[eA] = base + np.arange(cna) * SG
            if cb:
                eB = sedge[sbst:sbst + cb]
                pos[eB] = base + (cna + np.arange(cb)) * SG
                off[c, p, cna:cna + cb, col] = one
    return nodesAB, off, pos


def _stage_x(x, nodes):
    """x slice [NS, H] f32 -> bf16 [h=128, 2, G, m=128]: chunk (half, g)
    column m holds x of local node nodes[half, m, g]."""
    xb = x.astype(BF16)                       # [NS, H]
    sel = xb[nodes.reshape(2, 128, G)]        # [2, 128m, G, H]
    return np.ascontiguousarray(sel.transpose(3, 0, 2, 1))


def _run_with_retry(nc, in_maps, attempts=3):
    """The axon-tunneled devices occasionally report a transient
    NRT_EXEC_UNIT_UNRECOVERABLE; a spaced retry usually succeeds."""
    import time
    from concourse import bass_utils
    last = None
    for k in range(attempts):
        try:
            return bass_utils.run_bass_kernel_spmd(
                nc, in_maps, core_ids=list(range(N_CORES)))
        except Exception as e:  # noqa: BLE001 - device transient
            last = e
            time.sleep(3.0 * (k + 1))
    raise last


def kernel(x_src, x_dst, edge_label_index, weight, bias):
    x_src = np.ascontiguousarray(np.asarray(x_src, dtype=np.float32))
    x_dst = np.ascontiguousarray(np.asarray(x_dst, dtype=np.float32))
    idx = np.asarray(edge_label_index)
    i0 = idx[0].astype(np.int64)
    i1 = idx[1].astype(np.int64)
    wgt = np.asarray(weight, dtype=np.float32)
    b = np.asarray(bias, dtype=np.float32)

    if "l1" not in _CACHE:
        _CACHE["l1"] = _build_launch1()
    if "l2" not in _CACHE:
        _CACHE["l2"] = _build_launch2()
    nc1, nc2 = _CACHE["l1"], _CACHE["l2"]

    nodes0, off0, pos0 = _prep_side(i0, 0)
    nodes1, off1, pos1 = _prep_side(i1, 1)

    # w staged on partitions (K = h), one column per side
    wv = np.zeros((128, 2), BF16)
    wv[:, 0] = wgt[0, :HIDDEN].astype(BF16)
    wv[:, 1] = wgt[0, HIDDEN:].astype(BF16)

    in_maps1 = []
    for c in range(N_CORES):
        in_maps1.append({
            "xs": _stage_x(x_src[c * NS:(c + 1) * NS], nodes0[c]),
            "xd": _stage_x(x_dst[c * NS:(c + 1) * NS], nodes1[c]),
            "wv": wv,
            "biasr": np.full((128, 1), b[0], np.float32),
            "offb": np.concatenate([off0[c], off1[c]], axis=2),
        })
    res1 = _run_with_retry(nc1, in_maps1)
    GG = np.concatenate(
        [res1.results[c]["g01"].reshape(-1) for c in range(N_CORES)])

    # realign halves to edge order (host permutation only)
    a0 = np.zeros(N_CORES * E_OUT, BF16)
    a1 = np.zeros(N_CORES * E_OUT, BF16)
    v0 = GG[(i0 // NS) * SLOT + pos0]
    v1 = GG[(i1 // NS) * SLOT + pos1]
    for c in range(N_CORES):
        e0, e1 = c * PER, (c + 1) * PER
        a0[c * E_OUT:c * E_OUT + PER] = v0[e0:e1]
        a1[c * E_OUT:c * E_OUT + PER] = v1[e0:e1]

    in_maps2 = [{
        "a01": np.ascontiguousarray(np.stack([
            a0[c * E_OUT:(c + 1) * E_OUT].reshape(128, COLS),
            a1[c * E_OUT:(c + 1) * E_OUT].reshape(128, COLS)], axis=1)),
    } for c in range(N_CORES)]
    res2 = _run_with_retry(nc2, in_maps2)

    out = np.empty(N_EDGES, np.float32)
    for c in range(N_CORES):
        out[c * PER:(c + 1) * PER] = \
            res2.results[c]["o"].reshape(-1)[:PER].astype(np.float32)
    return out.reshape(N_EDGES, 1)


# revision 43
# speedup vs baseline: 1.2147x; 1.2147x over previous
"""EdgeDecoder kernel for 8 Trainium2 NeuronCores.

Math: out[e] = dot(x_src[i0[e]], w_src) + dot(x_dst[i1[e]], w_dst) + bias.
Rewritten as per-node scores s[n] = x_src[n]@w_src + bias, d[n] = x_dst[n]@w_dst,
then out[e] = s[i0[e]] + d[i1[e]].

Device pipeline (launch 1, per core, per side):
  - Host packs each core's ~250k edges into 128*G tiles of F=40 slots; a
    tile holds edges of at most TWO nodes (free pairing, ~98% fill).
  - Host stages x per tile-node as bf16 [h=128, half, g, m=128]: the two
    nodes of tile (p, g) sit in matmul chunk (half=0, g) and (half=1, g),
    column p.  Phase A is pure PE: 2*G chunk-stationary matmuls
    (lhsT = x^T chunk, rhs = w as [128,1]) land both per-tile endpoint
    scores in PSUM [128, 2, G] - already in window order, so there is no
    score table, no DRAM round-trip and no indirect gather at all.
  - One ACT copy (f32->bf16) -> W [128, 2, G]; dif = W1-W0 (DVE).
  - Per-edge value is a lerp  g = W0 + off*(W1-W0)  with host-shipped
    off in {0,1} (bf16 [128, F, G]): two bf16 DVE passes in 2x_1p mode.
Launch 2 adds the two host-realigned halves (device arithmetic only; the
host only permutes/casts between launches).
"""

import numpy as np
import ml_dtypes

BF16 = ml_dtypes.bfloat16

N_NODES = 100000
HIDDEN = 128
N_EDGES = 2000000
N_CORES = 8
NS = N_NODES // N_CORES         # 12500 nodes per core
F = 40                          # edge slots per tile
G = 51                          # tiles per partition per side (data needs 50)
SG = 2 * G                      # both sides' tile columns, s then d
NB = 8                          # PSUM bank rotation for matmul outputs
NCOL = [(G - k + NB - 1) // NB for k in range(NB)]  # cols per bank tile
OFFC = np.concatenate([[0], np.cumsum(NCOL)])[:NB]
# matmul emission index m (bank m%NB) -> device w column within the side
M2C = np.array([OFFC[m % NB] + m // NB for m in range(G)])
SLOT = 128 * F * SG             # g01 slots per core (both sides)
PER = N_EDGES // N_CORES        # 250000 edges per launch-2 core
COLS = (PER + 127) // 128       # 1954
E_OUT = COLS * 128              # 250112 padded launch-2 edges per core

_CACHE = {}


def _mybir():
    import concourse.mybir as mybir
    return mybir


def _build_launch1(reps=1):
    from contextlib import ExitStack
    import concourse.bacc as bacc
    import concourse.tile as tile
    mybir = _mybir()
    f32 = mybir.dt.float32
    bf16 = mybir.dt.bfloat16

    nc = bacc.Bacc("TRN2", debug=False, num_devices=N_CORES)
    xs = nc.dram_tensor("xs", [128, 2, G, 128], bf16, kind="ExternalInput")
    xd = nc.dram_tensor("xd", [128, 2, G, 128], bf16, kind="ExternalInput")
    wv = nc.dram_tensor("wv", [128, 2], bf16, kind="ExternalInput")
    biasr = nc.dram_tensor("biasr", [128, 1], f32, kind="ExternalInput")
    offb = nc.dram_tensor("offb", [128, F, SG], bf16, kind="ExternalInput")
    g01 = nc.dram_tensor("g01", [128, F, SG], bf16, kind="ExternalOutput")

    add = mybir.AluOpType.add
    mult = mybir.AluOpType.mult
    sub = mybir.AluOpType.subtract
    is_ge = mybir.AluOpType.is_ge
    XCH = 17  # g-columns per x-load DMA (3 loads per half)

    with tile.TileContext(nc) as tc:
        with tc.tile_pool(name="const", bufs=1) as cp, \
             tc.tile_pool(name="xload", bufs=4) as xp, \
             tc.tile_pool(name="work", bufs=2) as wp, \
             tc.tile_pool(name="psum", bufs=1, space="PSUM") as pp:

            wv_t = cp.tile([128, 2], bf16)
            nc.sync.dma_start(out=wv_t[:], in_=wv.ap()[:, :])
            bias_t = cp.tile([128, 1], f32, name="bias_t")
            nc.sync.dma_start(out=bias_t[:], in_=biasr.ap()[:, :])

            _loop = ExitStack()
            if reps > 1:
                _loop.enter_context(
                    tc.For_i(0, reps, 1,
                             hint_engines=(mybir.EngineType.PE,)))

            # scores for both sides land in one [128, 2, SG] tile; column
            # sidx*G + g is tile (.,g) of that side, row 'half' its A/B node
            w = wp.tile([128, 2, SG], bf16, name="w_t", tag="w")
            offt = wp.tile([128, F, SG], bf16, name="offt_t", tag="offt")
            nc.scalar.dma_start(out=offt[:], in_=offb.ap()[:, :, :])

            def side(x, wcol, sidx, nm, use_bias):
                # phase A: per-tile endpoint scores, in window order.
                # Matmul m writes PSUM bank m%NB, column m//NB: consecutive
                # matmuls never serialize on same-bank writeback.  Bank k
                # maps to the contiguous w columns [OFFC[k], OFFC[k]+NCOL[k])
                # (host stages everything in that renumbered column order).
                pst = [pp.tile([128, 2, NCOL[k]], f32, name=f"ps_{nm}{k}",
                               tag=f"ps{k}") for k in range(NB)]
                for half in range(2):
                    for c0 in range(0, G, XCH):
                        c1 = min(c0 + XCH, G)
                        xt = xp.tile([128, XCH, 128], bf16,
                                     name=f"xt_{nm}{half}{c0}", tag="xt")
                        nc.sync.dma_start(
                            out=xt[:, :c1 - c0, :],
                            in_=x.ap()[:, half, c0:c1, :])
                        for j in range(c1 - c0):
                            m = c0 + j
                            nc.tensor.matmul(
                                pst[m % NB][:, half, m // NB:m // NB + 1],
                                xt[:, j, :],
                                wv_t[:, wcol:wcol + 1])
                for k in range(NB):
                    o0 = sidx * G + int(OFFC[k])
                    nc.scalar.activation(
                        out=w[:, :, o0:o0 + NCOL[k]],
                        in_=pst[k][:, :, :],
                        func=mybir.ActivationFunctionType.Copy)
                if use_bias:
                    nc.vector.tensor_scalar_add(
                        out=w[:, :, sidx * G:(sidx + 1) * G],
                        in0=w[:, :, sidx * G:(sidx + 1) * G],
                        scalar1=bias_t[:, :])

            side(xs, 0, 0, "s", True)
            side(xd, 1, 1, "d", False)

            # phase B over both sides at once: lerp select with the
            # host-shipped step mask
            dif = wp.tile([128, SG], bf16, name="dif_t", tag="dif")
            nc.vector.tensor_tensor(
                out=dif[:], in0=w[:, 1, :], in1=w[:, 0, :], op=sub)
            prod = wp.tile([128, F, SG], bf16, name="prod_t", tag="prod")
            nc.vector.tensor_tensor(
                out=prod[:],
                in0=offt[:],
                in1=dif[:].rearrange("p g -> p () g").to_broadcast(
                    [128, F, SG]),
                op=mult)
            gt = wp.tile([128, F, SG], bf16, name="gt_t", tag="gt")
            nc.vector.tensor_tensor(
                out=gt[:],
                in0=prod[:],
                in1=w[:, 0, :].rearrange("p g -> p () g").to_broadcast(
                    [128, F, SG]),
                op=add)
            nc.scalar.dma_start(out=g01.ap()[:, :, :], in_=gt[:])
            _loop.close()

    nc.compile()
    return nc


def _build_launch2(reps=1):
    from contextlib import ExitStack
    import concourse.bacc as bacc
    import concourse.tile as tile
    mybir = _mybir()
    bf16 = mybir.dt.bfloat16

    nc = bacc.Bacc("TRN2", debug=False, num_devices=N_CORES)
    a01 = nc.dram_tensor("a01", [128, 2, COLS], bf16, kind="ExternalInput")
    o = nc.dram_tensor("o", [128, COLS], bf16, kind="ExternalOutput")
    with tile.TileContext(nc) as tc:
        with tc.tile_pool(name="io", bufs=3) as io:
            _loop = ExitStack()
            if reps > 1:
                _loop.enter_context(tc.For_i(0, reps, 1))
            step = 977
            for c0 in range(0, COLS, step):
                c1 = min(c0 + step, COLS)
                t0 = io.tile([128, 2, c1 - c0], bf16, name=f"t0_{c0}",
                             tag="t0")
                to = io.tile([128, c1 - c0], bf16, name=f"to_{c0}", tag="to")
                nc.sync.dma_start(out=t0[:], in_=a01.ap()[:, :, c0:c1])
                nc.vector.tensor_tensor(out=to[:], in0=t0[:, 0, :],
                                        in1=t0[:, 1, :],
                                        op=mybir.AluOpType.add)
                nc.scalar.dma_start(out=o.ap()[:, c0:c1], in_=to[:])
            _loop.close()
    nc.compile()
    return nc


def _prep_side(iarr, side):
    """Per-core: pack edges into F-slot tiles of at most 2 nodes each
    (big nodes split into full tiles; leftovers two-pointer paired).

    Returns nodesAB [CORES,2,128,G] i64 (local node per tile half,
    indexed by matmul emission index m), off [CORES,128,F,G] bf16
    (indexed by device column c=M2C[m]; 1.0 on slots holding the B
    node's edges), pos [E] i64 (slot of edge e in its core's combined
    g01 output: p*(F*SG) + f*SG + side*G + c)."""
    E = iarr.shape[0]
    nodesAB = np.zeros((N_CORES, 2, 128, G), np.int64)
    off = np.zeros((N_CORES, 128, F, G), BF16)
    pos = np.empty(E, np.int64)
    one = BF16(1.0)
    for c in range(N_CORES):
        sel = np.nonzero((iarr >= c * NS) & (iarr < (c + 1) * NS))[0]
        li = iarr[sel] - c * NS
        so = np.argsort(li, kind="stable")
        sli = li[so]
        sedge = sel[so]
        counts = np.bincount(sli, minlength=NS)
        starts = np.concatenate([[0], np.cumsum(counts)])
        # tiles: (nodeA, sliceA, nodeB, sliceB)
        tiles = []
        rem = []  # (count, node, start_index)
        for n in range(NS):
            cnt = int(counts[n])
            st = int(starts[n])
            nfull = cnt // F
            for k in range(nfull):
                tiles.append((n, st + k * F, F, n, 0, 0))
            r = cnt % F
            if r:
                rem.append((r, n, st + nfull * F))
        rem.sort()
        i, j = 0, len(rem) - 1
        while i <= j:
            ra, na, sa = rem[j]
            if i < j and ra + rem[i][0] <= F:
                rb, nb, sbst = rem[i]
                tiles.append((na, sa, ra, nb, sbst, rb))
                i += 1
                j -= 1
            else:
                tiles.append((na, sa, ra, na, 0, 0))
                j -= 1
        if len(tiles) > 128 * G:
            raise RuntimeError(
                f"tile capacity exceeded on core {c}: {len(tiles)}")
        for t, (na, sa, cna, nb, sbst, cb) in enumerate(tiles):
            p, m = t % 128, t // 128
            col = int(M2C[m])
            nodesAB[c, 0, p, m] = na
            nodesAB[c, 1, p, m] = nb
            base = p * (F * SG) + side * G + col
            eA = sedge[sa:sa + cna]
            pos[eA] = base + np.arange(cna) * SG
            if cb:
                eB = sedge[sbst:sbst + cb]
                pos[eB] = base + (cna + np.arange(cb)) * SG
                off[c, p, cna:cna + cb, col] = one
    return nodesAB, off, pos


def _stage_x(x, nodes):
    """x slice [NS, H] f32 -> bf16 [h=128, 2, G, m=128]: chunk (half, g)
    column m holds x of local node nodes[half, m, g]."""
    xb = x.astype(BF16)                       # [NS, H]
    sel = xb[nodes.reshape(2, 128, G)]        # [2, 128m, G, H]
    return np.ascontiguousarray(sel.transpose(3, 0, 2, 1))


def _run_with_retry(nc, in_maps, attempts=3):
    """The axon-tunneled devices occasionally report a transient
    NRT_EXEC_UNIT_UNRECOVERABLE; a spaced retry usually succeeds."""
    import time
    from concourse import bass_utils
    last = None
    for k in range(attempts):
        try:
            return bass_utils.run_bass_kernel_spmd(
                nc, in_maps, core_ids=list(range(N_CORES)))
        except Exception as e:  # noqa: BLE001 - device transient
            last = e
            time.sleep(3.0 * (k + 1))
    raise last


def kernel(x_src, x_dst, edge_label_index, weight, bias):
    x_src = np.ascontiguousarray(np.asarray(x_src, dtype=np.float32))
    x_dst = np.ascontiguousarray(np.asarray(x_dst, dtype=np.float32))
    idx = np.asarray(edge_label_index)
    i0 = idx[0].astype(np.int64)
    i1 = idx[1].astype(np.int64)
    wgt = np.asarray(weight, dtype=np.float32)
    b = np.asarray(bias, dtype=np.float32)

    if "l1" not in _CACHE:
        _CACHE["l1"] = _build_launch1()
    if "l2" not in _CACHE:
        _CACHE["l2"] = _build_launch2()
    nc1, nc2 = _CACHE["l1"], _CACHE["l2"]

    nodes0, off0, pos0 = _prep_side(i0, 0)
    nodes1, off1, pos1 = _prep_side(i1, 1)

    # w staged on partitions (K = h), one column per side
    wv = np.zeros((128, 2), BF16)
    wv[:, 0] = wgt[0, :HIDDEN].astype(BF16)
    wv[:, 1] = wgt[0, HIDDEN:].astype(BF16)

    in_maps1 = []
    for c in range(N_CORES):
        in_maps1.append({
            "xs": _stage_x(x_src[c * NS:(c + 1) * NS], nodes0[c]),
            "xd": _stage_x(x_dst[c * NS:(c + 1) * NS], nodes1[c]),
            "wv": wv,
            "biasr": np.full((128, 1), b[0], np.float32),
            "offb": np.concatenate([off0[c], off1[c]], axis=2),
        })
    res1 = _run_with_retry(nc1, in_maps1)
    GG = np.concatenate(
        [res1.results[c]["g01"].reshape(-1) for c in range(N_CORES)])

    # realign halves to edge order (host permutation only)
    a0 = np.zeros(N_CORES * E_OUT, BF16)
    a1 = np.zeros(N_CORES * E_OUT, BF16)
    v0 = GG[(i0 // NS) * SLOT + pos0]
    v1 = GG[(i1 // NS) * SLOT + pos1]
    for c in range(N_CORES):
        e0, e1 = c * PER, (c + 1) * PER
        a0[c * E_OUT:c * E_OUT + PER] = v0[e0:e1]
        a1[c * E_OUT:c * E_OUT + PER] = v1[e0:e1]

    in_maps2 = [{
        "a01": np.ascontiguousarray(np.stack([
            a0[c * E_OUT:(c + 1) * E_OUT].reshape(128, COLS),
            a1[c * E_OUT:(c + 1) * E_OUT].reshape(128, COLS)], axis=1)),
    } for c in range(N_CORES)]
    res2 = _run_with_retry(nc2, in_maps2)

    out = np.empty(N_EDGES, np.float32)
    for c in range(N_CORES):
        out[c * PER:(c + 1) * PER] = \
            res2.results[c]["o"].reshape(-1)[:PER].astype(np.float32)
    return out.reshape(N_EDGES, 1)


# revision 44
# speedup vs baseline: 1.2736x; 1.0485x over previous
"""EdgeDecoder kernel for 8 Trainium2 NeuronCores.

Math: out[e] = dot(x_src[i0[e]], w_src) + dot(x_dst[i1[e]], w_dst) + bias.
Rewritten as per-node scores s[n] = x_src[n]@w_src + bias, d[n] = x_dst[n]@w_dst,
then out[e] = s[i0[e]] + d[i1[e]].

Device pipeline (launch 1, per core, per side):
  - Host packs each core's ~250k edges into 128*G tiles of F=40 slots; a
    tile holds edges of at most TWO nodes (free pairing, ~98% fill).
  - Host stages x per tile-node as bf16 [h=128, half, g, m=128]: the two
    nodes of tile (p, g) sit in matmul chunk (half=0, g) and (half=1, g),
    column p.  Phase A is pure PE: 2*G chunk-stationary matmuls
    (lhsT = x^T chunk, rhs = w as [128,1]) land both per-tile endpoint
    scores in PSUM [128, 2, G] - already in window order, so there is no
    score table, no DRAM round-trip and no indirect gather at all.
  - One ACT copy (f32->bf16) -> W [128, 2, G]; dif = W1-W0 (DVE).
  - Per-edge value is a lerp  g = W0 + off*(W1-W0)  with host-shipped
    off in {0,1} (bf16 [128, F, G]): two bf16 DVE passes in 2x_1p mode.
Launch 2 adds the two host-realigned halves (device arithmetic only; the
host only permutes/casts between launches).
"""

import numpy as np
import ml_dtypes

BF16 = ml_dtypes.bfloat16

N_NODES = 100000
HIDDEN = 128
N_EDGES = 2000000
N_CORES = 8
NS = N_NODES // N_CORES         # 12500 nodes per core
F = 40                          # edge slots per tile
G = 51                          # tiles per partition per side (data needs 50)
SG = 2 * G                      # both sides' tile columns, s then d
NB = 8                          # PSUM bank rotation for matmul outputs
NCOL = [(G - k + NB - 1) // NB for k in range(NB)]  # cols per bank tile
OFFC = np.concatenate([[0], np.cumsum(NCOL)])[:NB]
# matmul emission index m (bank m%NB) -> device w column within the side
M2C = np.array([OFFC[m % NB] + m // NB for m in range(G)])
SLOT = 128 * F * SG             # g01 slots per core (both sides)
PER = N_EDGES // N_CORES        # 250000 edges per launch-2 core
COLS = (PER + 127) // 128       # 1954
E_OUT = COLS * 128              # 250112 padded launch-2 edges per core

_CACHE = {}


def _mybir():
    import concourse.mybir as mybir
    return mybir


def _build_launch1(reps=1):
    from contextlib import ExitStack
    import concourse.bacc as bacc
    import concourse.tile as tile
    mybir = _mybir()
    f32 = mybir.dt.float32
    bf16 = mybir.dt.bfloat16

    nc = bacc.Bacc("TRN2", debug=False, num_devices=N_CORES)
    xs = nc.dram_tensor("xs", [128, 2, G, 128], bf16, kind="ExternalInput")
    xd = nc.dram_tensor("xd", [128, 2, G, 128], bf16, kind="ExternalInput")
    wv = nc.dram_tensor("wv", [128, 2], bf16, kind="ExternalInput")
    biasr = nc.dram_tensor("biasr", [128, 1], f32, kind="ExternalInput")
    offb = nc.dram_tensor("offb", [128, F, SG], bf16, kind="ExternalInput")
    g01 = nc.dram_tensor("g01", [128, F, SG], bf16, kind="ExternalOutput")

    add = mybir.AluOpType.add
    mult = mybir.AluOpType.mult
    sub = mybir.AluOpType.subtract
    XCH = 17  # g-columns per x-load DMA (3 loads per half)

    with tile.TileContext(nc) as tc:
        with tc.tile_pool(name="const", bufs=1) as cp, \
             tc.tile_pool(name="xload", bufs=4) as xp, \
             tc.tile_pool(name="work", bufs=2) as wp, \
             tc.tile_pool(name="psum", bufs=1, space="PSUM") as pp:

            wv_t = cp.tile([128, 2], bf16)
            nc.sync.dma_start(out=wv_t[:], in_=wv.ap()[:, :])
            bias_t = cp.tile([128, 1], f32, name="bias_t")
            nc.sync.dma_start(out=bias_t[:], in_=biasr.ap()[:, :])

            _loop = ExitStack()
            if reps > 1:
                _loop.enter_context(
                    tc.For_i(0, reps, 1,
                             hint_engines=(mybir.EngineType.PE,)))

            # scores for both sides land in one [128, 2, SG] tile; column
            # sidx*G + g is tile (.,g) of that side, row 'half' its A/B node
            w = wp.tile([128, 2, SG], bf16, name="w_t", tag="w")
            offt = wp.tile([128, F, SG], bf16, name="offt_t", tag="offt")
            nc.scalar.dma_start(out=offt[:], in_=offb.ap()[:, :, :])

            def side(x, wcol, sidx, nm, use_bias):
                # phase A: per-tile endpoint scores, in window order.
                # Matmul m writes PSUM bank m%NB, column m//NB: consecutive
                # matmuls never serialize on same-bank writeback.  Bank k
                # maps to the contiguous w columns [OFFC[k], OFFC[k]+NCOL[k])
                # (host stages everything in that renumbered column order).
                pst = [pp.tile([128, 2, NCOL[k]], f32, name=f"ps_{nm}{k}",
                               tag=f"ps{k}") for k in range(NB)]
                for half in range(2):
                    for c0 in range(0, G, XCH):
                        c1 = min(c0 + XCH, G)
                        xt = xp.tile([128, XCH, 128], bf16,
                                     name=f"xt_{nm}{half}{c0}", tag="xt")
                        nc.sync.dma_start(
                            out=xt[:, :c1 - c0, :],
                            in_=x.ap()[:, half, c0:c1, :])
                        for j in range(c1 - c0):
                            m = c0 + j
                            nc.tensor.matmul(
                                pst[m % NB][:, half, m // NB:m // NB + 1],
                                xt[:, j, :],
                                wv_t[:, wcol:wcol + 1])
                for k in range(NB):
                    o0 = sidx * G + int(OFFC[k])
                    nc.scalar.activation(
                        out=w[:, :, o0:o0 + NCOL[k]],
                        in_=pst[k][:, :, :],
                        func=mybir.ActivationFunctionType.Copy)
                if use_bias:
                    nc.vector.tensor_scalar_add(
                        out=w[:, :, sidx * G:(sidx + 1) * G],
                        in0=w[:, :, sidx * G:(sidx + 1) * G],
                        scalar1=bias_t[:, :])

            side(xs, 0, 0, "s", True)
            side(xd, 1, 1, "d", False)

            # phase B over both sides at once: lerp select with the
            # host-shipped step mask
            dif = wp.tile([128, SG], bf16, name="dif_t", tag="dif")
            nc.vector.tensor_tensor(
                out=dif[:], in0=w[:, 1, :], in1=w[:, 0, :], op=sub)
            prod = wp.tile([128, F, SG], bf16, name="prod_t", tag="prod")
            nc.vector.tensor_tensor(
                out=prod[:],
                in0=offt[:],
                in1=dif[:].rearrange("p g -> p () g").to_broadcast(
                    [128, F, SG]),
                op=mult)
            gt = wp.tile([128, F, SG], bf16, name="gt_t", tag="gt")
            nc.vector.tensor_tensor(
                out=gt[:],
                in0=prod[:],
                in1=w[:, 0, :].rearrange("p g -> p () g").to_broadcast(
                    [128, F, SG]),
                op=add)
            nc.scalar.dma_start(out=g01.ap()[:, :, :], in_=gt[:])
            _loop.close()

    nc.compile()
    return nc


def _build_launch2(reps=1):
    from contextlib import ExitStack
    import concourse.bacc as bacc
    import concourse.tile as tile
    mybir = _mybir()
    bf16 = mybir.dt.bfloat16

    nc = bacc.Bacc("TRN2", debug=False, num_devices=N_CORES)
    a01 = nc.dram_tensor("a01", [128, 2, COLS], bf16, kind="ExternalInput")
    o = nc.dram_tensor("o", [128, COLS], bf16, kind="ExternalOutput")
    with tile.TileContext(nc) as tc:
        with tc.tile_pool(name="io", bufs=3) as io:
            _loop = ExitStack()
            if reps > 1:
                _loop.enter_context(tc.For_i(0, reps, 1))
            step = 977
            for c0 in range(0, COLS, step):
                c1 = min(c0 + step, COLS)
                t0 = io.tile([128, 2, c1 - c0], bf16, name=f"t0_{c0}",
                             tag="t0")
                to = io.tile([128, c1 - c0], bf16, name=f"to_{c0}", tag="to")
                nc.sync.dma_start(out=t0[:], in_=a01.ap()[:, :, c0:c1])
                nc.vector.tensor_tensor(out=to[:], in0=t0[:, 0, :],
                                        in1=t0[:, 1, :],
                                        op=mybir.AluOpType.add)
                nc.scalar.dma_start(out=o.ap()[:, c0:c1], in_=to[:])
            _loop.close()
    nc.compile()
    return nc


def _prep_side(iarr, side):
    """Per-core: pack edges into F-slot tiles of at most 2 nodes each
    (big nodes split into full tiles; leftovers two-pointer paired).

    Returns nodesAB [CORES,2,128,G] i64 (local node per tile half,
    indexed by matmul emission index m), off [CORES,128,F,G] bf16
    (indexed by device column c=M2C[m]; 1.0 on slots holding the B
    node's edges), pos [E] i64 (slot of edge e in its core's combined
    g01 output: p*(F*SG) + f*SG + side*G + c)."""
    E = iarr.shape[0]
    nodesAB = np.zeros((N_CORES, 2, 128, G), np.int64)
    off = np.zeros((N_CORES, 128, F, G), BF16)
    pos = np.empty(E, np.int64)
    one = BF16(1.0)
    for c in range(N_CORES):
        sel = np.nonzero((iarr >= c * NS) & (iarr < (c + 1) * NS))[0]
        li = iarr[sel] - c * NS
        so = np.argsort(li, kind="stable")
        sli = li[so]
        sedge = sel[so]
        counts = np.bincount(sli, minlength=NS)
        starts = np.concatenate([[0], np.cumsum(counts)])
        # tiles: (nodeA, sliceA, nodeB, sliceB)
        tiles = []
        rem = []  # (count, node, start_index)
        for n in range(NS):
            cnt = int(counts[n])
            st = int(starts[n])
            nfull = cnt // F
            for k in range(nfull):
                tiles.append((n, st + k * F, F, n, 0, 0))
            r = cnt % F
            if r:
                rem.append((r, n, st + nfull * F))
        rem.sort()
        i, j = 0, len(rem) - 1
        while i <= j:
            ra, na, sa = rem[j]
            if i < j and ra + rem[i][0] <= F:
                rb, nb, sbst = rem[i]
                tiles.append((na, sa, ra, nb, sbst, rb))
                i += 1
                j -= 1
            else:
                tiles.append((na, sa, ra, na, 0, 0))
                j -= 1
        if len(tiles) > 128 * G:
            raise RuntimeError(
                f"tile capacity exceeded on core {c}: {len(tiles)}")
        for t, (na, sa, cna, nb, sbst, cb) in enumerate(tiles):
            p, m = t % 128, t // 128
            col = int(M2C[m])
            nodesAB[c, 0, p, m] = na
            nodesAB[c, 1, p, m] = nb
            base = p * (F * SG) + side * G + col
            eA = sedge[sa:sa + cna]
            pos[eA] = base + np.arange(cna) * SG
            if cb:
                eB = sedge[sbst:sbst + cb]
                pos[eB] = base + (cna + np.arange(cb)) * SG
                off[c, p, cna:cna + cb, col] = one
    return nodesAB, off, pos


def _stage_x(x, nodes):
    """x slice [NS, H] f32 -> bf16 [h=128, 2, G, m=128]: chunk (half, g)
    column m holds x of local node nodes[half, m, g]."""
    xb = x.astype(BF16)                       # [NS, H]
    sel = xb[nodes.reshape(2, 128, G)]        # [2, 128m, G, H]
    return np.ascontiguousarray(sel.transpose(3, 0, 2, 1))


def _run_with_retry(nc, in_maps, attempts=3):
    """The axon-tunneled devices occasionally report a transient
    NRT_EXEC_UNIT_UNRECOVERABLE; a spaced retry usually succeeds."""
    import time
    from concourse import bass_utils
    last = None
    for k in range(attempts):
        try:
            return bass_utils.run_bass_kernel_spmd(
                nc, in_maps, core_ids=list(range(N_CORES)))
        except Exception as e:  # noqa: BLE001 - device transient
            last = e
            time.sleep(3.0 * (k + 1))
    raise last


def kernel(x_src, x_dst, edge_label_index, weight, bias):
    x_src = np.ascontiguousarray(np.asarray(x_src, dtype=np.float32))
    x_dst = np.ascontiguousarray(np.asarray(x_dst, dtype=np.float32))
    idx = np.asarray(edge_label_index)
    i0 = idx[0].astype(np.int64)
    i1 = idx[1].astype(np.int64)
    wgt = np.asarray(weight, dtype=np.float32)
    b = np.asarray(bias, dtype=np.float32)

    if "l1" not in _CACHE:
        _CACHE["l1"] = _build_launch1()
    if "l2" not in _CACHE:
        _CACHE["l2"] = _build_launch2()
    nc1, nc2 = _CACHE["l1"], _CACHE["l2"]

    nodes0, off0, pos0 = _prep_side(i0, 0)
    nodes1, off1, pos1 = _prep_side(i1, 1)

    # w staged on partitions (K = h), one column per side
    wv = np.zeros((128, 2), BF16)
    wv[:, 0] = wgt[0, :HIDDEN].astype(BF16)
    wv[:, 1] = wgt[0, HIDDEN:].astype(BF16)

    in_maps1 = []
    for c in range(N_CORES):
        in_maps1.append({
            "xs": _stage_x(x_src[c * NS:(c + 1) * NS], nodes0[c]),
            "xd": _stage_x(x_dst[c * NS:(c + 1) * NS], nodes1[c]),
            "wv": wv,
            "biasr": np.full((128, 1), b[0], np.float32),
            "offb": np.concatenate([off0[c], off1[c]], axis=2),
        })
    res1 = _run_with_retry(nc1, in_maps1)
    GG = np.concatenate(
        [res1.results[c]["g01"].reshape(-1) for c in range(N_CORES)])

    # realign halves to edge order (host permutation only)
    a0 = np.zeros(N_CORES * E_OUT, BF16)
    a1 = np.zeros(N_CORES * E_OUT, BF16)
    v0 = GG[(i0 // NS) * SLOT + pos0]
    v1 = GG[(i1 // NS) * SLOT + pos1]
    for c in range(N_CORES):
        e0, e1 = c * PER, (c + 1) * PER
        a0[c * E_OUT:c * E_OUT + PER] = v0[e0:e1]
        a1[c * E_OUT:c * E_OUT + PER] = v1[e0:e1]

    in_maps2 = [{
        "a01": np.ascontiguousarray(np.stack([
            a0[c * E_OUT:(c + 1) * E_OUT].reshape(128, COLS),
            a1[c * E_OUT:(c + 1) * E_OUT].reshape(128, COLS)], axis=1)),
    } for c in range(N_CORES)]
    res2 = _run_with_retry(nc2, in_maps2)

    out = np.empty(N_EDGES, np.float32)
    for c in range(N_CORES):
        out[c * PER:(c + 1) * PER] = \
            res2.results[c]["o"].reshape(-1)[:PER].astype(np.float32)
    return out.reshape(N_EDGES, 1)


# revision 48
# speedup vs baseline: 1.5026x; 1.1798x over previous
"""EdgeDecoder kernel for 8 Trainium2 NeuronCores.

Math: out[e] = dot(x_src[i0[e]], w_src) + dot(x_dst[i1[e]], w_dst) + bias.
Rewritten as per-node scores s[n] = x_src[n]@w_src + bias, d[n] = x_dst[n]@w_dst,
then out[e] = s[i0[e]] + d[i1[e]].

Device pipeline (launch 1, per core, per side):
  - Host packs each core's ~250k edges into 128*G tiles of F=40 slots; a
    tile holds edges of at most TWO nodes (free pairing, ~98% fill).
  - Host stages x per tile-node as bf16 [h=128, half, g, m=128]: the two
    nodes of tile (p, g) sit in matmul chunk (half=0, g) and (half=1, g),
    column p.  Phase A is pure PE: 2*G chunk-stationary matmuls
    (lhsT = x^T chunk, rhs = w as [128,1]) land both per-tile endpoint
    scores in PSUM [128, 2, G] - already in window order, so there is no
    score table, no DRAM round-trip and no indirect gather at all.
  - One ACT copy (f32->bf16) -> W [128, 2, G]; dif = W1-W0 (DVE).
  - Per-edge value is a lerp  g = W0 + off*(W1-W0)  with host-shipped
    off in {0,1} (bf16 [128, F, G]): two bf16 DVE passes in 2x_1p mode.
Launch 2 adds the two host-realigned halves (device arithmetic only; the
host only permutes/casts between launches).
"""

import numpy as np
import ml_dtypes

BF16 = ml_dtypes.bfloat16

N_NODES = 100000
HIDDEN = 128
N_EDGES = 2000000
N_CORES = 8
NS = N_NODES // N_CORES         # 12500 nodes per core
F = 40                          # edge slots per tile
G = 50                          # tiles per partition per side (data needs 49.5)
SG = 2 * G                      # both sides' tile columns, s then d
NB = 8                          # PSUM bank rotation for matmul outputs
NCOL = [(G - k + NB - 1) // NB for k in range(NB)]  # cols per bank tile
OFFC = np.concatenate([[0], np.cumsum(NCOL)])[:NB]
# matmul emission index m (bank m%NB) -> device w column within the side
M2C = np.array([OFFC[m % NB] + m // NB for m in range(G)])
SLOT = 128 * F * SG             # g01 slots per core (both sides)
PER = N_EDGES // N_CORES        # 250000 edges per launch-2 core
COLS = (PER + 127) // 128       # 1954
E_OUT = COLS * 128              # 250112 padded launch-2 edges per core

_CACHE = {}


def _mybir():
    import concourse.mybir as mybir
    return mybir


def _build_launch1(reps=1):
    from contextlib import ExitStack
    import concourse.bacc as bacc
    import concourse.tile as tile
    mybir = _mybir()
    f32 = mybir.dt.float32
    bf16 = mybir.dt.bfloat16

    nc = bacc.Bacc("TRN2", debug=False, num_devices=N_CORES)
    xs = nc.dram_tensor("xs", [128, 2, G, 128], bf16, kind="ExternalInput")
    xd = nc.dram_tensor("xd", [128, 2, G, 128], bf16, kind="ExternalInput")
    wv = nc.dram_tensor("wv", [128, 2], bf16, kind="ExternalInput")
    biasr = nc.dram_tensor("biasr", [128, 1], f32, kind="ExternalInput")
    offb = nc.dram_tensor("offb", [128, F, SG], bf16, kind="ExternalInput")
    g01 = nc.dram_tensor("g01", [128, F, SG], bf16, kind="ExternalOutput")

    add = mybir.AluOpType.add
    mult = mybir.AluOpType.mult
    sub = mybir.AluOpType.subtract
    XCH = 17  # g-columns per x-load DMA (3 loads per half)

    with tile.TileContext(nc) as tc:
        with tc.tile_pool(name="const", bufs=1) as cp, \
             tc.tile_pool(name="xload", bufs=4) as xp, \
             tc.tile_pool(name="work", bufs=2) as wp, \
             tc.tile_pool(name="psum", bufs=1, space="PSUM") as pp:

            wv_t = cp.tile([128, 2], bf16)
            nc.sync.dma_start(out=wv_t[:], in_=wv.ap()[:, :])
            bias_t = cp.tile([128, 1], f32, name="bias_t")
            nc.sync.dma_start(out=bias_t[:], in_=biasr.ap()[:, :])

            _loop = ExitStack()
            if reps > 1:
                _loop.enter_context(
                    tc.For_i(0, reps, 1,
                             hint_engines=(mybir.EngineType.PE,)))

            # scores for both sides land in one [128, 2, SG] tile; column
            # sidx*G + g is tile (.,g) of that side, row 'half' its A/B node
            w = wp.tile([128, 2, SG], bf16, name="w_t", tag="w")
            offt = wp.tile([128, F, SG], bf16, name="offt_t", tag="offt")
            nc.scalar.dma_start(out=offt[:], in_=offb.ap()[:, :, :])

            def side(x, wcol, sidx, nm, use_bias):
                # phase A: per-tile endpoint scores, in window order.
                # Matmul m writes PSUM bank m%NB, column m//NB: consecutive
                # matmuls never serialize on same-bank writeback.  Bank k
                # maps to the contiguous w columns [OFFC[k], OFFC[k]+NCOL[k])
                # (host stages everything in that renumbered column order).
                pst = [pp.tile([128, 2, NCOL[k]], f32, name=f"ps_{nm}{k}",
                               tag=f"ps{k}") for k in range(NB)]
                for half in range(2):
                    for c0 in range(0, G, XCH):
                        c1 = min(c0 + XCH, G)
                        xt = xp.tile([128, XCH, 128], bf16,
                                     name=f"xt_{nm}{half}{c0}", tag="xt")
                        nc.sync.dma_start(
                            out=xt[:, :c1 - c0, :],
                            in_=x.ap()[:, half, c0:c1, :])
                        for j in range(c1 - c0):
                            m = c0 + j
                            nc.tensor.matmul(
                                pst[m % NB][:, half, m // NB:m // NB + 1],
                                xt[:, j, :],
                                wv_t[:, wcol:wcol + 1])
                for k in range(NB):
                    o0 = sidx * G + int(OFFC[k])
                    nc.scalar.activation(
                        out=w[:, :, o0:o0 + NCOL[k]],
                        in_=pst[k][:, :, :],
                        func=mybir.ActivationFunctionType.Copy)
                if use_bias:
                    nc.vector.tensor_scalar_add(
                        out=w[:, :, sidx * G:(sidx + 1) * G],
                        in0=w[:, :, sidx * G:(sidx + 1) * G],
                        scalar1=bias_t[:, :])

            # phase B: lerp select with the host-shipped step mask, split
            # per side so the s-side chain overlaps the d-side matmuls
            dif = wp.tile([128, SG], bf16, name="dif_t", tag="dif")
            prod = wp.tile([128, F, SG], bf16, name="prod_t", tag="prod")
            gt = wp.tile([128, F, SG], bf16, name="gt_t", tag="gt")

            def select(sidx):
                lo, hi = sidx * G, (sidx + 1) * G
                nc.vector.tensor_tensor(
                    out=dif[:, lo:hi], in0=w[:, 1, lo:hi],
                    in1=w[:, 0, lo:hi], op=sub)
                nc.vector.tensor_tensor(
                    out=prod[:, :, lo:hi],
                    in0=offt[:, :, lo:hi],
                    in1=dif[:, lo:hi].rearrange(
                        "p g -> p () g").to_broadcast([128, F, G]),
                    op=mult)
                nc.vector.tensor_tensor(
                    out=gt[:, :, lo:hi],
                    in0=prod[:, :, lo:hi],
                    in1=w[:, 0, lo:hi].rearrange(
                        "p g -> p () g").to_broadcast([128, F, G]),
                    op=add)

            side(xs, 0, 0, "s", True)
            select(0)
            side(xd, 1, 1, "d", False)
            select(1)
            nc.scalar.dma_start(out=g01.ap()[:, :, :], in_=gt[:])
            _loop.close()

    nc.compile()
    return nc


def _build_launch2(reps=1):
    from contextlib import ExitStack
    import concourse.bacc as bacc
    import concourse.tile as tile
    mybir = _mybir()
    bf16 = mybir.dt.bfloat16

    nc = bacc.Bacc("TRN2", debug=False, num_devices=N_CORES)
    a01 = nc.dram_tensor("a01", [128, 2, COLS], bf16, kind="ExternalInput")
    o = nc.dram_tensor("o", [128, COLS], bf16, kind="ExternalOutput")
    with tile.TileContext(nc) as tc:
        with tc.tile_pool(name="io", bufs=3) as io:
            _loop = ExitStack()
            if reps > 1:
                _loop.enter_context(tc.For_i(0, reps, 1))
            step = 977
            for c0 in range(0, COLS, step):
                c1 = min(c0 + step, COLS)
                t0 = io.tile([128, 2, c1 - c0], bf16, name=f"t0_{c0}",
                             tag="t0")
                to = io.tile([128, c1 - c0], bf16, name=f"to_{c0}", tag="to")
                nc.sync.dma_start(out=t0[:], in_=a01.ap()[:, :, c0:c1])
                nc.vector.tensor_tensor(out=to[:], in0=t0[:, 0, :],
                                        in1=t0[:, 1, :],
                                        op=mybir.AluOpType.add)
                nc.scalar.dma_start(out=o.ap()[:, c0:c1], in_=to[:])
            _loop.close()
    nc.compile()
    return nc


def _prep_side(iarr, side):
    """Per-core: pack edges into F-slot tiles of at most 2 nodes each
    (big nodes split into full tiles; leftovers two-pointer paired).

    Returns nodesAB [CORES,2,128,G] i64 (local node per tile half,
    indexed by matmul emission index m), off [CORES,128,F,G] bf16
    (indexed by device column c=M2C[m]; 1.0 on slots holding the B
    node's edges), pos [E] i64 (slot of edge e in its core's combined
    g01 output: p*(F*SG) + f*SG + side*G + c)."""
    E = iarr.shape[0]
    nodesAB = np.zeros((N_CORES, 2, 128, G), np.int64)
    off = np.zeros((N_CORES, 128, F, G), BF16)
    pos = np.empty(E, np.int64)
    one = BF16(1.0)
    for c in range(N_CORES):
        sel = np.nonzero((iarr >= c * NS) & (iarr < (c + 1) * NS))[0]
        li = iarr[sel] - c * NS
        so = np.argsort(li, kind="stable")
        sli = li[so]
        sedge = sel[so]
        counts = np.bincount(sli, minlength=NS)
        starts = np.concatenate([[0], np.cumsum(counts)])
        # tiles: (nodeA, sliceA, nodeB, sliceB)
        tiles = []
        rem = []  # (count, node, start_index)
        for n in range(NS):
            cnt = int(counts[n])
            st = int(starts[n])
            nfull = cnt // F
            for k in range(nfull):
                tiles.append((n, st + k * F, F, n, 0, 0))
            r = cnt % F
            if r:
                rem.append((r, n, st + nfull * F))
        rem.sort()
        i, j = 0, len(rem) - 1
        while i <= j:
            ra, na, sa = rem[j]
            if i < j and ra + rem[i][0] <= F:
                rb, nb, sbst = rem[i]
                tiles.append((na, sa, ra, nb, sbst, rb))
                i += 1
                j -= 1
            else:
                tiles.append((na, sa, ra, na, 0, 0))
                j -= 1
        if len(tiles) > 128 * G:
            raise RuntimeError(
                f"tile capacity exceeded on core {c}: {len(tiles)}")
        for t, (na, sa, cna, nb, sbst, cb) in enumerate(tiles):
            p, m = t % 128, t // 128
            col = int(M2C[m])
            nodesAB[c, 0, p, m] = na
            nodesAB[c, 1, p, m] = nb
            base = p * (F * SG) + side * G + col
            eA = sedge[sa:sa + cna]
            pos[eA] = base + np.arange(cna) * SG
            if cb:
                eB = sedge[sbst:sbst + cb]
                pos[eB] = base + (cna + np.arange(cb)) * SG
                off[c, p, cna:cna + cb, col] = one
    return nodesAB, off, pos


def _stage_x(x, nodes):
    """x slice [NS, H] f32 -> bf16 [h=128, 2, G, m=128]: chunk (half, g)
    column m holds x of local node nodes[half, m, g]."""
    xb = x.astype(BF16)                       # [NS, H]
    sel = xb[nodes.reshape(2, 128, G)]        # [2, 128m, G, H]
    return np.ascontiguousarray(sel.transpose(3, 0, 2, 1))


def _run_with_retry(nc, in_maps, attempts=3):
    """The axon-tunneled devices occasionally report a transient
    NRT_EXEC_UNIT_UNRECOVERABLE; a spaced retry usually succeeds."""
    import time
    from concourse import bass_utils
    last = None
    for k in range(attempts):
        try:
            return bass_utils.run_bass_kernel_spmd(
                nc, in_maps, core_ids=list(range(N_CORES)))
        except Exception as e:  # noqa: BLE001 - device transient
            last = e
            time.sleep(3.0 * (k + 1))
    raise last


def kernel(x_src, x_dst, edge_label_index, weight, bias):
    x_src = np.ascontiguousarray(np.asarray(x_src, dtype=np.float32))
    x_dst = np.ascontiguousarray(np.asarray(x_dst, dtype=np.float32))
    idx = np.asarray(edge_label_index)
    i0 = idx[0].astype(np.int64)
    i1 = idx[1].astype(np.int64)
    wgt = np.asarray(weight, dtype=np.float32)
    b = np.asarray(bias, dtype=np.float32)

    if "l1" not in _CACHE:
        _CACHE["l1"] = _build_launch1()
    if "l2" not in _CACHE:
        _CACHE["l2"] = _build_launch2()
    nc1, nc2 = _CACHE["l1"], _CACHE["l2"]

    nodes0, off0, pos0 = _prep_side(i0, 0)
    nodes1, off1, pos1 = _prep_side(i1, 1)

    # w staged on partitions (K = h), one column per side
    wv = np.zeros((128, 2), BF16)
    wv[:, 0] = wgt[0, :HIDDEN].astype(BF16)
    wv[:, 1] = wgt[0, HIDDEN:].astype(BF16)

    in_maps1 = []
    for c in range(N_CORES):
        in_maps1.append({
            "xs": _stage_x(x_src[c * NS:(c + 1) * NS], nodes0[c]),
            "xd": _stage_x(x_dst[c * NS:(c + 1) * NS], nodes1[c]),
            "wv": wv,
            "biasr": np.full((128, 1), b[0], np.float32),
            "offb": np.concatenate([off0[c], off1[c]], axis=2),
        })
    res1 = _run_with_retry(nc1, in_maps1)
    GG = np.concatenate(
        [res1.results[c]["g01"].reshape(-1) for c in range(N_CORES)])

    # realign halves to edge order (host permutation only)
    a0 = np.zeros(N_CORES * E_OUT, BF16)
    a1 = np.zeros(N_CORES * E_OUT, BF16)
    v0 = GG[(i0 // NS) * SLOT + pos0]
    v1 = GG[(i1 // NS) * SLOT + pos1]
    for c in range(N_CORES):
        e0, e1 = c * PER, (c + 1) * PER
        a0[c * E_OUT:c * E_OUT + PER] = v0[e0:e1]
        a1[c * E_OUT:c * E_OUT + PER] = v1[e0:e1]

    in_maps2 = [{
        "a01": np.ascontiguousarray(np.stack([
            a0[c * E_OUT:(c + 1) * E_OUT].reshape(128, COLS),
            a1[c * E_OUT:(c + 1) * E_OUT].reshape(128, COLS)], axis=1)),
    } for c in range(N_CORES)]
    res2 = _run_with_retry(nc2, in_maps2)

    out = np.empty(N_EDGES, np.float32)
    for c in range(N_CORES):
        out[c * PER:(c + 1) * PER] = \
            res2.results[c]["o"].reshape(-1)[:PER].astype(np.float32)
    return out.reshape(N_EDGES, 1)


# revision 50
# speedup vs baseline: 1.6712x; 1.1122x over previous
"""EdgeDecoder kernel for 8 Trainium2 NeuronCores.

Math: out[e] = dot(x_src[i0[e]], w_src) + dot(x_dst[i1[e]], w_dst) + bias.
Rewritten as per-node scores s[n] = x_src[n]@w_src + bias, d[n] = x_dst[n]@w_dst,
then out[e] = s[i0[e]] + d[i1[e]].

Device pipeline (launch 1, per core, per side):
  - Host packs each core's ~250k edges into 128*G tiles of F=40 slots; a
    tile holds edges of at most TWO nodes (free pairing, ~98% fill).
  - Host stages x per tile-node as bf16 [h=128, half, g, m=128]: the two
    nodes of tile (p, g) sit in matmul chunk (half=0, g) and (half=1, g),
    column p.  Phase A is pure PE: 2*G chunk-stationary matmuls
    (lhsT = x^T chunk, rhs = w as [128,1]) land both per-tile endpoint
    scores in PSUM [128, 2, G] - already in window order, so there is no
    score table, no DRAM round-trip and no indirect gather at all.
  - One ACT copy (f32->bf16) -> W [128, 2, G]; dif = W1-W0 (DVE).
  - Per-edge value is a lerp  g = W0 + off*(W1-W0)  with host-shipped
    off in {0,1} (bf16 [128, F, G]): two bf16 DVE passes in 2x_1p mode.
Launch 2 adds the two host-realigned halves (device arithmetic only; the
host only permutes/casts between launches).
"""

import numpy as np
import ml_dtypes

BF16 = ml_dtypes.bfloat16

N_NODES = 100000
HIDDEN = 128
N_EDGES = 2000000
N_CORES = 8
NS = N_NODES // N_CORES         # 12500 nodes per core
F = 40                          # edge slots per tile
G = 50                          # tiles per partition per side (data needs 49.5)
SG = 2 * G                      # both sides' tile columns, s then d
NB = 8                          # PSUM bank rotation for matmul outputs
NCOL = [(G - k + NB - 1) // NB for k in range(NB)]  # cols per bank tile
OFFC = np.concatenate([[0], np.cumsum(NCOL)])[:NB]
# matmul emission index m (bank m%NB) -> device w column within the side
M2C = np.array([OFFC[m % NB] + m // NB for m in range(G)])
SLOT = 128 * F * SG             # g01 slots per core (both sides)
PER = N_EDGES // N_CORES        # 250000 edges per launch-2 core
COLS = (PER + 127) // 128       # 1954
E_OUT = COLS * 128              # 250112 padded launch-2 edges per core

_CACHE = {}


def _mybir():
    import concourse.mybir as mybir
    return mybir


def _build_launch1(reps=1):
    from contextlib import ExitStack
    import concourse.bacc as bacc
    import concourse.tile as tile
    mybir = _mybir()
    f32 = mybir.dt.float32
    bf16 = mybir.dt.bfloat16

    nc = bacc.Bacc("TRN2", debug=False, num_devices=N_CORES)
    xs = nc.dram_tensor("xs", [128, 2, G, 128], bf16, kind="ExternalInput")
    xd = nc.dram_tensor("xd", [128, 2, G, 128], bf16, kind="ExternalInput")
    wv = nc.dram_tensor("wv", [128, 2], bf16, kind="ExternalInput")
    biasr = nc.dram_tensor("biasr", [128, 1], f32, kind="ExternalInput")
    offb = nc.dram_tensor("offb", [128, F, SG], bf16, kind="ExternalInput")
    g01 = nc.dram_tensor("g01", [128, F, SG], bf16, kind="ExternalOutput")

    add = mybir.AluOpType.add
    mult = mybir.AluOpType.mult
    sub = mybir.AluOpType.subtract
    XCH = 13  # g-columns per x-load DMA (4 loads per half)

    with tile.TileContext(nc) as tc:
        with tc.tile_pool(name="const", bufs=1) as cp, \
             tc.tile_pool(name="xload", bufs=6) as xp, \
             tc.tile_pool(name="work", bufs=2) as wp, \
             tc.tile_pool(name="psum", bufs=1, space="PSUM") as pp:

            wv_t = cp.tile([128, 2], bf16)
            nc.sync.dma_start(out=wv_t[:], in_=wv.ap()[:, :])
            bias_t = cp.tile([128, 1], f32, name="bias_t")
            nc.sync.dma_start(out=bias_t[:], in_=biasr.ap()[:, :])

            _loop = ExitStack()
            if reps > 1:
                _loop.enter_context(
                    tc.For_i(0, reps, 1,
                             hint_engines=(mybir.EngineType.PE,)))

            # scores for both sides land in one [128, 2, SG] tile; column
            # sidx*G + g is tile (.,g) of that side, row 'half' its A/B node
            w = wp.tile([128, 2, SG], bf16, name="w_t", tag="w")
            offt = wp.tile([128, F, SG], bf16, name="offt_t", tag="offt")
            nc.scalar.dma_start(out=offt[:], in_=offb.ap()[:, :, :])

            def side(x, wcol, sidx, nm, use_bias):
                # phase A: per-tile endpoint scores, in window order.
                # Matmul m writes PSUM bank m%NB, column m//NB: consecutive
                # matmuls never serialize on same-bank writeback.  Bank k
                # maps to the contiguous w columns [OFFC[k], OFFC[k]+NCOL[k])
                # (host stages everything in that renumbered column order).
                pst = [pp.tile([128, 2, NCOL[k]], f32, name=f"ps_{nm}{k}",
                               tag=f"ps{k}") for k in range(NB)]
                for half in range(2):
                    for c0 in range(0, G, XCH):
                        c1 = min(c0 + XCH, G)
                        xt = xp.tile([128, XCH, 128], bf16,
                                     name=f"xt_{nm}{half}{c0}", tag="xt")
                        nc.sync.dma_start(
                            out=xt[:, :c1 - c0, :],
                            in_=x.ap()[:, half, c0:c1, :])
                        for j in range(c1 - c0):
                            m = c0 + j
                            nc.tensor.matmul(
                                pst[m % NB][:, half, m // NB:m // NB + 1],
                                xt[:, j, :],
                                wv_t[:, wcol:wcol + 1])
                for k in range(NB):
                    o0 = sidx * G + int(OFFC[k])
                    nc.scalar.activation(
                        out=w[:, :, o0:o0 + NCOL[k]],
                        in_=pst[k][:, :, :],
                        func=mybir.ActivationFunctionType.Copy)
                if use_bias:
                    nc.vector.tensor_scalar_add(
                        out=w[:, :, sidx * G:(sidx + 1) * G],
                        in0=w[:, :, sidx * G:(sidx + 1) * G],
                        scalar1=bias_t[:, :])

            # phase B: lerp select with the host-shipped step mask, split
            # per side so the s-side chain overlaps the d-side matmuls
            dif = wp.tile([128, SG], bf16, name="dif_t", tag="dif")
            prod = wp.tile([128, F, SG], bf16, name="prod_t", tag="prod")
            gt = wp.tile([128, F, SG], bf16, name="gt_t", tag="gt")

            def select(sidx):
                lo, hi = sidx * G, (sidx + 1) * G
                nc.vector.tensor_tensor(
                    out=dif[:, lo:hi], in0=w[:, 1, lo:hi],
                    in1=w[:, 0, lo:hi], op=sub)
                nc.vector.tensor_tensor(
                    out=prod[:, :, lo:hi],
                    in0=offt[:, :, lo:hi],
                    in1=dif[:, lo:hi].rearrange(
                        "p g -> p () g").to_broadcast([128, F, G]),
                    op=mult)
                nc.vector.tensor_tensor(
                    out=gt[:, :, lo:hi],
                    in0=prod[:, :, lo:hi],
                    in1=w[:, 0, lo:hi].rearrange(
                        "p g -> p () g").to_broadcast([128, F, G]),
                    op=add)

            side(xs, 0, 0, "s", True)
            select(0)
            side(xd, 1, 1, "d", False)
            select(1)
            nc.scalar.dma_start(out=g01.ap()[:, :, :], in_=gt[:])
            _loop.close()

    nc.compile()
    return nc


def _build_launch2(reps=1):
    from contextlib import ExitStack
    import concourse.bacc as bacc
    import concourse.tile as tile
    mybir = _mybir()
    bf16 = mybir.dt.bfloat16

    nc = bacc.Bacc("TRN2", debug=False, num_devices=N_CORES)
    a01 = nc.dram_tensor("a01", [128, 2, COLS], bf16, kind="ExternalInput")
    o = nc.dram_tensor("o", [128, COLS], bf16, kind="ExternalOutput")
    with tile.TileContext(nc) as tc:
        with tc.tile_pool(name="io", bufs=3) as io:
            _loop = ExitStack()
            if reps > 1:
                _loop.enter_context(tc.For_i(0, reps, 1))
            step = 977
            for c0 in range(0, COLS, step):
                c1 = min(c0 + step, COLS)
                t0 = io.tile([128, 2, c1 - c0], bf16, name=f"t0_{c0}",
                             tag="t0")
                to = io.tile([128, c1 - c0], bf16, name=f"to_{c0}", tag="to")
                nc.sync.dma_start(out=t0[:], in_=a01.ap()[:, :, c0:c1])
                nc.vector.tensor_tensor(out=to[:], in0=t0[:, 0, :],
                                        in1=t0[:, 1, :],
                                        op=mybir.AluOpType.add)
                nc.scalar.dma_start(out=o.ap()[:, c0:c1], in_=to[:])
            _loop.close()
    nc.compile()
    return nc


def _prep_side(iarr, side):
    """Per-core: pack edges into F-slot tiles of at most 2 nodes each
    (big nodes split into full tiles; leftovers two-pointer paired).

    Returns nodesAB [CORES,2,128,G] i64 (local node per tile half,
    indexed by matmul emission index m), off [CORES,128,F,G] bf16
    (indexed by device column c=M2C[m]; 1.0 on slots holding the B
    node's edges), pos [E] i64 (slot of edge e in its core's combined
    g01 output: p*(F*SG) + f*SG + side*G + c)."""
    E = iarr.shape[0]
    nodesAB = np.zeros((N_CORES, 2, 128, G), np.int64)
    off = np.zeros((N_CORES, 128, F, G), BF16)
    pos = np.empty(E, np.int64)
    one = BF16(1.0)
    for c in range(N_CORES):
        sel = np.nonzero((iarr >= c * NS) & (iarr < (c + 1) * NS))[0]
        li = iarr[sel] - c * NS
        so = np.argsort(li, kind="stable")
        sli = li[so]
        sedge = sel[so]
        counts = np.bincount(sli, minlength=NS)
        starts = np.concatenate([[0], np.cumsum(counts)])
        # tiles: (nodeA, sliceA, nodeB, sliceB)
        tiles = []
        rem = []  # (count, node, start_index)
        for n in range(NS):
            cnt = int(counts[n])
            st = int(starts[n])
            nfull = cnt // F
            for k in range(nfull):
                tiles.append((n, st + k * F, F, n, 0, 0))
            r = cnt % F
            if r:
                rem.append((r, n, st + nfull * F))
        rem.sort()
        i, j = 0, len(rem) - 1
        while i <= j:
            ra, na, sa = rem[j]
            if i < j and ra + rem[i][0] <= F:
                rb, nb, sbst = rem[i]
                tiles.append((na, sa, ra, nb, sbst, rb))
                i += 1
                j -= 1
            else:
                tiles.append((na, sa, ra, na, 0, 0))
                j -= 1
        if len(tiles) > 128 * G:
            raise RuntimeError(
                f"tile capacity exceeded on core {c}: {len(tiles)}")
        for t, (na, sa, cna, nb, sbst, cb) in enumerate(tiles):
            p, m = t % 128, t // 128
            col = int(M2C[m])
            nodesAB[c, 0, p, m] = na
            nodesAB[c, 1, p, m] = nb
            base = p * (F * SG) + side * G + col
            eA = sedge[sa:sa + cna]
            pos[eA] = base + np.arange(cna) * SG
            if cb:
                eB = sedge[sbst:sbst + cb]
                pos[eB] = base + (cna + np.arange(cb)) * SG
                off[c, p, cna:cna + cb, col] = one
    return nodesAB, off, pos


def _stage_x(x, nodes):
    """x slice [NS, H] f32 -> bf16 [h=128, 2, G, m=128]: chunk (half, g)
    column m holds x of local node nodes[half, m, g]."""
    xb = x.astype(BF16)                       # [NS, H]
    sel = xb[nodes.reshape(2, 128, G)]        # [2, 128m, G, H]
    return np.ascontiguousarray(sel.transpose(3, 0, 2, 1))


def _run_with_retry(nc, in_maps, attempts=3):
    """The axon-tunneled devices occasionally report a transient
    NRT_EXEC_UNIT_UNRECOVERABLE; a spaced retry usually succeeds."""
    import time
    from concourse import bass_utils
    last = None
    for k in range(attempts):
        try:
            return bass_utils.run_bass_kernel_spmd(
                nc, in_maps, core_ids=list(range(N_CORES)))
        except Exception as e:  # noqa: BLE001 - device transient
            last = e
            time.sleep(3.0 * (k + 1))
    raise last


def kernel(x_src, x_dst, edge_label_index, weight, bias):
    x_src = np.ascontiguousarray(np.asarray(x_src, dtype=np.float32))
    x_dst = np.ascontiguousarray(np.asarray(x_dst, dtype=np.float32))
    idx = np.asarray(edge_label_index)
    i0 = idx[0].astype(np.int64)
    i1 = idx[1].astype(np.int64)
    wgt = np.asarray(weight, dtype=np.float32)
    b = np.asarray(bias, dtype=np.float32)

    if "l1" not in _CACHE:
        _CACHE["l1"] = _build_launch1()
    if "l2" not in _CACHE:
        _CACHE["l2"] = _build_launch2()
    nc1, nc2 = _CACHE["l1"], _CACHE["l2"]

    nodes0, off0, pos0 = _prep_side(i0, 0)
    nodes1, off1, pos1 = _prep_side(i1, 1)

    # w staged on partitions (K = h), one column per side
    wv = np.zeros((128, 2), BF16)
    wv[:, 0] = wgt[0, :HIDDEN].astype(BF16)
    wv[:, 1] = wgt[0, HIDDEN:].astype(BF16)

    in_maps1 = []
    for c in range(N_CORES):
        in_maps1.append({
            "xs": _stage_x(x_src[c * NS:(c + 1) * NS], nodes0[c]),
            "xd": _stage_x(x_dst[c * NS:(c + 1) * NS], nodes1[c]),
            "wv": wv,
            "biasr": np.full((128, 1), b[0], np.float32),
            "offb": np.concatenate([off0[c], off1[c]], axis=2),
        })
    res1 = _run_with_retry(nc1, in_maps1)
    GG = np.concatenate(
        [res1.results[c]["g01"].reshape(-1) for c in range(N_CORES)])

    # realign halves to edge order (host permutation only)
    a0 = np.zeros(N_CORES * E_OUT, BF16)
    a1 = np.zeros(N_CORES * E_OUT, BF16)
    v0 = GG[(i0 // NS) * SLOT + pos0]
    v1 = GG[(i1 // NS) * SLOT + pos1]
    for c in range(N_CORES):
        e0, e1 = c * PER, (c + 1) * PER
        a0[c * E_OUT:c * E_OUT + PER] = v0[e0:e1]
        a1[c * E_OUT:c * E_OUT + PER] = v1[e0:e1]

    in_maps2 = [{
        "a01": np.ascontiguousarray(np.stack([
            a0[c * E_OUT:(c + 1) * E_OUT].reshape(128, COLS),
            a1[c * E_OUT:(c + 1) * E_OUT].reshape(128, COLS)], axis=1)),
    } for c in range(N_CORES)]
    res2 = _run_with_retry(nc2, in_maps2)

    out = np.empty(N_EDGES, np.float32)
    for c in range(N_CORES):
        out[c * PER:(c + 1) * PER] = \
            res2.results[c]["o"].reshape(-1)[:PER].astype(np.float32)
    return out.reshape(N_EDGES, 1)
